# revision 36
# baseline (speedup 1.0000x reference)
"""Trainium2 Bass kernel for nn_Mnist_lmdSplineKAN.

Sharding: 2D -- batch x4 (256 rows/core) by head-group x2 (5 heads = 320
out cols/core). All 8 cores do identical-shape work.

Math: the uniform-grid cubic B-spline basis is rewritten in the truncated
power basis,  f_j(z) = (1/6) sum_r (-1)^r C(4,r) (z+3-j-r)_+^3  with z=5x.
Splitting each (z-m)_+^3 into a smooth cubic (folded into the weights on
the host) plus a bounded one-sided cube leaves just 8 device feature
planes: d, d^2, d^3 (d = z-2.5), S1=(1-z)_+^3, S2=(2-z)_+^3, R3=(z-3)_+^3,
R4=(z-4)_+^3, and silu(x). The constant term becomes a bias row added via
a rank-1 ones-matmul. Features are fp16 stationary; weights fp16 moving;
PSUM fp32.

I=784 is tiled as 6 full chunks of 128 + 16 leftover rows; the leftover
rows x 8 planes pack into one K=128 matmul via an SBUF->SBUF repack.
Weights stream plane-major on the sync HWDGE queue in matmul consumption
order; warmup matmuls ramp the PE p-state before the real wavefront.
"""
import sys, types
import numpy as np

B, I, O, H = 1024, 784, 64, 10
NC, BG, OG = 8, 4, 2
BC = B // BG          # 256 batch rows per core
HOC = (H // OG) * O   # 320 output cols per core
D2C = (H // OG) * 32  # 160 hidden cols per core
NCH = 6               # full 128-row input chunks
PL = 16               # leftover input rows (chunk 6)
NP = 7                # feature planes: xc, xc^2, xc^3, R3, R4, S1, S2
NWARM = 16

# plane order: d, d2, silu, d3, R3, R4, S1, S2
C5 = np.array([1., -4., 6., -4., 1.]) / 6.0


def _tables():
    polyc = np.zeros((8, 4))
    tapS = np.zeros((8, 2))
    tapR = np.zeros((8, 2))
    for j in range(8):
        for r in range(5):
            m = j - 3 + r
            cc = C5[r]
            if m >= 5:
                continue
            if m in (3, 4):
                tapR[j, m - 3] += cc
            else:
                a = 2.5 - m
                polyc[j] += cc * np.array([a**3, 3 * a**2, 3 * a, 1.0])
                if m in (1, 2):
                    tapS[j, m - 1] += cc
    return polyc, tapS, tapR


def _install_ntff_hook():
    if "antenv.axon_hooks" in sys.modules:
        return
    try:
        import antenv
        mod = types.ModuleType("antenv.axon_hooks")
        _h = [None]
        mod.set_axon_ntff_profile_hook = lambda h: _h.__setitem__(0, h)
        mod.get_axon_ntff_profile_hook = lambda: _h[0]
        sys.modules["antenv.axon_hooks"] = mod
        antenv.axon_hooks = mod
        from trn_agent_boot.trn_boot import _ntff_profile_via_ctypes
        h = _ntff_profile_via_ctypes("/opt/axon/libaxon_pjrt.so")
        if h is not None:
            mod.set_axon_ntff_profile_hook(h)
    except Exception:
        pass


_CACHE = {}


def _build():
    if "nc" in _CACHE:
        return _CACHE["nc"]
    import concourse.bacc as bacc
    import concourse.bass as bass
    import concourse.tile as tile
    from concourse import mybir
    from contextlib import ExitStack

    f32, f16 = mybir.dt.float32, mybir.dt.float16
    ALU = mybir.AluOpType
    AF = mybir.ActivationFunctionType

    nc = bacc.Bacc("TRN2", target_bir_lowering=False, debug=False)
    x_d = nc.dram_tensor("x", (128, 7 * BC), f16, kind="ExternalInput").ap()
    WROW = NCH * HOC                       # 1920 elems per partition per plane
    w_d = nc.dram_tensor("w", (NP * 128 * WROW + NP * PL * HOC,), f16,
                         kind="ExternalInput").ap()
    b_d = nc.dram_tensor("brow", (1, HOC + D2C), f16, kind="ExternalInput").ap()
    cf16_d = nc.dram_tensor("cf16", (128, 3 * D2C + 128), f16,
                            kind="ExternalInput").ap()
    cf32_d = nc.dram_tensor("cf32", (128, D2C + 5), f32,
                            kind="ExternalInput").ap()
    out_d = nc.dram_tensor("out", (BC, 5), f32, kind="ExternalOutput").ap()

    with tile.TileContext(nc) as tc, ExitStack() as ctx:
        sb = ctx.enter_context(tc.tile_pool(name="sb", bufs=1))
        ps = ctx.enter_context(tc.tile_pool(name="ps", bufs=1, space="PSUM"))

        # ---- DMAs: sync HWDGE = bias row + x (features depend on x);
        #      gpsimd SWDGE = weight planes in consumption order, then
        #      packed chunk-6 and tail consts ----
        brow = sb.tile([1, HOC + D2C], f16, tag="brow")
        nc.sync.dma_start(brow[:], b_d)
        # x split: first 4 chunks on the (early-starting) sync HWDGE queue,
        # last 3 on SWDGE ahead of the weights; flat APs = big descriptors
        xt = sb.tile([128, 7, BC], f16, tag="xt")
        XSPL = 4 * BC
        nc.sync.dma_start(xt[:].rearrange("p c b -> p (c b)")[:, 0:XSPL],
                          x_d[:, 0:XSPL])
        wAll = sb.tile([128, NP, NCH, HOC], f16, tag="wAll")
        # first half of plane 0 ahead of x's tail so the PE can start the
        # wavefront as soon as x lands
        HROW = WROW // 2
        nc.gpsimd.dma_start(
            wAll[:, 0, 0:3, :].rearrange("p c o -> p (c o)"),
            bass.AP(tensor=w_d.tensor, offset=0, ap=[[WROW, 128], [1, HROW]]))
        nc.gpsimd.dma_start(xt[:].rearrange("p c b -> p (c b)")[:, XSPL:],
                            x_d[:, XSPL:])
        nc.gpsimd.dma_start(
            wAll[:, 0, 3:6, :].rearrange("p c o -> p (c o)"),
            bass.AP(tensor=w_d.tensor, offset=HROW,
                    ap=[[WROW, 128], [1, HROW]]))
        for p in range(1, NP):
            src = bass.AP(tensor=w_d.tensor, offset=p * 128 * WROW,
                          ap=[[WROW, 128], [1, WROW]])
            nc.gpsimd.dma_start(
                wAll[:, p].rearrange("p c o -> p (c o)"), src)
        w6t = sb.tile([NP * PL, HOC], f16, tag="w6t")
        src6 = bass.AP(tensor=w_d.tensor, offset=NP * 128 * WROW,
                       ap=[[HOC, NP * PL], [1, HOC]])
        nc.gpsimd.dma_start(w6t[:], src6)

        cf16 = sb.tile([128, 3 * D2C + 128], f16, tag="cf16")
        nc.sync.dma_start(cf16[:], cf16_d)
        w1p = cf16[:, 0:3 * D2C].rearrange("p (k d) -> p k d", d=D2C)
        idt = cf16[:, 3 * D2C:]
        cf32 = sb.tile([128, D2C + 5], f32, tag="cf32")
        nc.sync.dma_start(cf32[:], cf32_d)
        w2b = cf32[:, 0:D2C]
        b2b = cf32[:, D2C:]

        ones = sb.tile([1, HOC], f16, tag="ones")
        nc.vector.memset(ones[:], 1.0)

        # force ACT tables to load during the DMA-wait window
        tl = sb.tile([1, 4], f16, tag="tl")
        for fn in (AF.Square, AF.Relu, AF.Tanh):
            nc.scalar.activation(tl[0:1, 0:1], ones[0:1, 0:1], fn)

        # ---- feature planes; xc = x - 0.5 comes pre-centered from host ----
        # plane order: 0:xc 1:xc^2 2:xc^3 3:R3 4:R4 5:S1 6:S2 where
        # R3=(x-0.6)+^3, R4=(x-0.8)+^3, S1=(0.2-x)+^3, S2=(0.4-x)+^3
        fall = sb.tile([128, NP - 1, 7, BC], f16, tag="fall")
        x2 = xt[:].rearrange("p c b -> p (c b)")

        def pl(p):
            if p == 0:
                return x2
            return fall[:, p - 1].rearrange("p c b -> p (c b)")

        def plc(p, c, bt):
            if p == 0:
                return xt[:, c, bt * 128:(bt + 1) * 128]
            return fall[:, p - 1, c, bt * 128:(bt + 1) * 128]

        def T(tag):
            return sb.tile([128, 7 * BC], f16, tag=tag, name=tag)

        bm3 = sb.tile([128, 1], f32, tag="bm3")
        nc.vector.memset(bm3[:], -0.3)
        bm1 = sb.tile([128, 1], f32, tag="bm1")
        nc.vector.memset(bm1[:], -0.1)
        s1 = T("s1"); s2 = T("s2"); r3 = T("r3"); r4 = T("r4")
        q1 = T("q1"); q2 = T("q2"); q3 = T("q3"); q4 = T("q4")
        # ACT: xc^2, s1=(0.2-x)+, s2=(0.4-x)+, s2^2
        nc.scalar.activation(pl(1), x2, AF.Square)
        nc.scalar.activation(s1[:], x2, AF.Relu, bias=bm3[:], scale=-1.0)
        nc.scalar.activation(s2[:], x2, AF.Relu, bias=bm1[:], scale=-1.0)
        nc.scalar.activation(q2[:], s2[:], AF.Square)
        # DVE: r3/r4 relus, xc^3, squares, cubes
        nc.vector.tensor_scalar(r3[:], x2, -0.1, 0.0, op0=ALU.add, op1=ALU.max)
        nc.vector.tensor_scalar(r4[:], x2, -0.3, 0.0, op0=ALU.add, op1=ALU.max)
        nc.vector.tensor_tensor(pl(2), pl(1), x2, op=ALU.mult)
        nc.vector.tensor_tensor(q3[:], r3[:], r3[:], op=ALU.mult)
        nc.vector.tensor_tensor(pl(3), q3[:], r3[:], op=ALU.mult)
        nc.vector.tensor_tensor(q4[:], r4[:], r4[:], op=ALU.mult)
        nc.vector.tensor_tensor(pl(4), q4[:], r4[:], op=ALU.mult)
        nc.vector.tensor_tensor(q1[:], s1[:], s1[:], op=ALU.mult)
        nc.vector.tensor_tensor(pl(5), q1[:], s1[:], op=ALU.mult)
        nc.vector.tensor_tensor(pl(6), q2[:], s2[:], op=ALU.mult)

        # ---- chunk-6 pack: 16 rows x 7 planes -> one K=112 tile ----
        f6 = sb.tile([NP * PL, BC], f16, tag="f6")
        nc.gpsimd.dma_start(f6[0:PL, :], xt[0:PL, 6, :])
        for p in range(1, NP):
            nc.gpsimd.dma_start(f6[p * PL:(p + 1) * PL, :],
                                fall[0:PL, p - 1, 6, :])

        # ---- matmuls ----
        # warmups depend only on the ones-memset, so the PE starts its
        # p-state ramp as soon as the engines come up (~7us before the
        # first weight plane lands)
        wu = ps.tile([128, HOC], f32, tag="wu")
        for k in range(NWARM):
            nc.tensor.matmul(wu[:], ones[0:1, 0:128], ones[:],
                             start=True, stop=True)

        y = [ps.tile([128, HOC], f32, tag=f"y{bt}", name=f"y{bt}")
             for bt in range(2)]
        for bt in range(2):
            nc.tensor.matmul(y[bt][:], ones[0:1, 0:128], brow[0:1, 0:HOC],
                             start=True, stop=False)
        for p in range(NP - 1):
            for c in range(NCH):
                for bt in range(2):
                    nc.tensor.matmul(y[bt][:], plc(p, c, bt),
                                     wAll[:, p, c, :], start=False, stop=False)
        # last plane + packed chunk-6: all of bt0 first so its PSUM bank
        # closes early and the bt0 tail overlaps bt1's matmuls
        for bt in range(2):
            for c in range(NCH):
                nc.tensor.matmul(y[bt][:], plc(NP - 1, c, bt),
                                 wAll[:, NP - 1, c, :], start=False, stop=False)
            nc.tensor.matmul(y[bt][:], f6[:, bt * 128:(bt + 1) * 128],
                             w6t[:], start=False, stop=True)

        # ---- tail per batch-tile: tanh, transpose, blockdiag MLP ----
        lgs = sb.tile([128, 2, 5], f32, tag="lgs")
        for bt in range(2):
            h1 = sb.tile([128, HOC], f16, tag=f"h1{bt}", name=f"h1{bt}")
            nc.scalar.activation(h1[:], y[bt][:], AF.Tanh)
            sts = []
            for k in range(3):
                kk = 128 if k < 2 else 64
                pt = ps.tile([128, 128], f16, tag=f"pt{k}",
                             name=f"pt{bt}{k}")
                nc.tensor.transpose(pt[0:kk, :], h1[:, k * 128:k * 128 + kk],
                                    idt)
                st = sb.tile([128, 128], f16, tag=f"st{bt}{k}",
                             name=f"st{bt}{k}")
                nc.vector.tensor_copy(st[0:kk, :], pt[0:kk, :])
                sts.append(st)
            ps2 = ps.tile([128, D2C], f32, tag=f"ps2{bt}", name=f"ps2{bt}")
            nc.tensor.matmul(ps2[:], ones[0:1, 0:128], brow[0:1, HOC:],
                             start=True, stop=False)
            for k in range(3):
                kk = 128 if k < 2 else 64
                nc.tensor.matmul(ps2[:], sts[k][0:kk, :], w1p[0:kk, k, :],
                                 start=False, stop=(k == 2))
            h2 = sb.tile([128, D2C], f32, tag=f"h2{bt}", name=f"h2{bt}")
            nc.scalar.activation(h2[:], ps2[:], AF.Tanh)
            prod = sb.tile([128, D2C], f32, tag=f"prod{bt}", name=f"prod{bt}")
            nc.vector.tensor_tensor(prod[:], h2[:], w2b, op=ALU.mult)
            red = sb.tile([128, 5], f32, tag=f"red{bt}", name=f"red{bt}")
            nc.vector.tensor_reduce(
                red[:], prod[:].rearrange("p (h d) -> p h d", d=32),
                axis=mybir.AxisListType.X, op=ALU.add)
            nc.vector.tensor_tensor(lgs[:, bt, :], red[:], b2b, op=ALU.add)
        # single out DMA: src (p, bt, col) -> dram row bt*128+p
        dst = bass.AP(tensor=out_d.tensor, offset=0,
                      ap=[[5, 128], [128 * 5, 2], [1, 5]])
        nc.sync.dma_start(dst, lgs[:])

    nc.compile()
    _CACHE["nc"] = nc
    return nc


def _prep_inputs(x, coef, scale_base, scale_sp, lmd, W1, b1, W2, b2):
    polyc, tapS, tapR = _tables()
    xf = np.asarray(x, np.float32).reshape(B, I)

    coef = np.asarray(coef, np.float64)
    eff = coef * np.asarray(scale_sp, np.float64)[..., None] \
        * np.asarray(lmd, np.float64)[:, :, None, None]        # (H, I, O, 8)
    W = eff.transpose(1, 3, 0, 2).reshape(I, 8, H * O)         # (I, 8, 640)
    sbl = (np.asarray(scale_base, np.float64)
           * np.asarray(lmd, np.float64)[:, :, None]
           ).transpose(1, 0, 2).reshape(I, H * O)

    # silu(x) lies (to ~1e-6) in the span of the 8-fn spline basis: fit it
    # and fold sbl * beta into the plane weights -- no silu plane on device
    g = np.linspace(0.0, 1.0, 4097)[:-1]
    gc = g - 0.5
    phi = np.stack([np.ones_like(g), gc, gc**2, gc**3,
                    np.maximum(g - 0.6, 0)**3, np.maximum(g - 0.8, 0)**3,
                    np.maximum(0.2 - g, 0)**3, np.maximum(0.4 - g, 0)**3], 1)
    beta = np.linalg.lstsq(phi, g / (1 + np.exp(-g)), rcond=None)[0]

    # fold: device plane order xc, xc^2, xc^3, R3', R4', S1', S2'
    # (d = 5*xc, so d-basis folds scale by 5^s; cubes by 125)
    Wp = np.empty((I, NP, H * O))
    Wp[:, 0] = 5.0 * np.einsum('j,ijo->io', polyc[:, 1], W) + beta[1] * sbl
    Wp[:, 1] = 25.0 * np.einsum('j,ijo->io', polyc[:, 2], W) + beta[2] * sbl
    Wp[:, 2] = 125.0 * np.einsum('j,ijo->io', polyc[:, 3], W) + beta[3] * sbl
    Wp[:, 3] = 125.0 * np.einsum('j,ijo->io', tapR[:, 0], W) + beta[4] * sbl
    Wp[:, 4] = 125.0 * np.einsum('j,ijo->io', tapR[:, 1], W) + beta[5] * sbl
    Wp[:, 5] = 125.0 * np.einsum('j,ijo->io', tapS[:, 0], W) + beta[6] * sbl
    Wp[:, 6] = 125.0 * np.einsum('j,ijo->io', tapS[:, 1], W) + beta[7] * sbl
    bias_full = np.einsum('j,ijo->o', polyc[:, 0], W) \
        + beta[0] * sbl.sum(0)                                 # (640,)

    W1 = np.asarray(W1, np.float64)
    W2 = np.asarray(W2, np.float64).reshape(H * 32)
    b1 = np.asarray(b1, np.float64).reshape(H * 32)
    b2 = np.asarray(b2, np.float64).reshape(H)

    per_og = []
    for og in range(OG):
        hs = slice(og * HOC, (og + 1) * HOC)
        # weight stream: 8 plane pieces [128, 6*320] then packed chunk-6
        pieces = []
        for p in range(NP):
            blk = Wp[0:NCH * 128, p, hs].reshape(NCH, 128, HOC)
            pieces.append(np.ascontiguousarray(
                blk.transpose(1, 0, 2)).reshape(-1))
        w6 = np.zeros((NP * PL, HOC))
        for p in range(NP):
            w6[p * PL:(p + 1) * PL] = Wp[NCH * 128:I, p, hs]
        pieces.append(np.ascontiguousarray(w6).reshape(-1))
        wdev = np.concatenate(pieces).astype(np.float16)

        brow = np.zeros((1, HOC + D2C))
        brow[0, 0:HOC] = bias_full[hs]
        brow[0, HOC:] = b1[og * D2C:(og + 1) * D2C]
        brow = brow.astype(np.float16)

        w1bd = np.zeros((HOC, D2C))
        for hl in range(H // OG):
            w1bd[hl * O:(hl + 1) * O, hl * 32:(hl + 1) * 32] = W1[og * (H // OG) + hl]
        w1dev = np.zeros((128, 3, D2C))
        w1dev[:, 0] = w1bd[0:128]
        w1dev[:, 1] = w1bd[128:256]
        w1dev[0:64, 2] = w1bd[256:HOC]
        cf16 = np.concatenate([w1dev.reshape(128, 3 * D2C),
                               np.eye(128)], 1).astype(np.float16)
        cf32 = np.concatenate([
            np.broadcast_to(W2[og * D2C:(og + 1) * D2C], (128, D2C)),
            np.broadcast_to(b2[og * 5:(og + 1) * 5], (128, 5))],
            1).astype(np.float32)
        per_og.append((wdev, brow, cf16, cf32))

    in_maps = []
    for core in range(NC):
        bg, og = core % BG, core // BG
        xs = (xf[bg * BC:(bg + 1) * BC].T - 0.5).astype(np.float16)  # (784, 256)
        xdev = np.zeros((7, 128, BC), np.float16)
        xdev.reshape(7 * 128, BC)[0:I] = xs
        xdev = np.ascontiguousarray(xdev.transpose(1, 0, 2)).reshape(128, 7 * BC)
        wdev, brow, cf16, cf32 = per_og[og]
        in_maps.append({"x": xdev, "w": wdev, "brow": brow,
                        "cf16": cf16, "cf32": cf32})
    return in_maps


def run(inputs, trace=False, tmpdir=None):
    _install_ntff_hook()
    from concourse.bass_utils import run_bass_kernel_spmd
    nc = _build()
    in_maps = _prep_inputs(**inputs)
    res = run_bass_kernel_spmd(nc, in_maps, core_ids=list(range(NC)),
                               trace=trace, tmpdir=tmpdir)
    out = np.empty((B, H), np.float32)
    for core in range(NC):
        bg, og = core % BG, core // BG
        out[bg * BC:(bg + 1) * BC, og * 5:(og + 1) * 5] = res.results[core]["out"]
    return out, res


def kernel(**inputs):
    out, _ = run(inputs)
    return out


# revision 37
# speedup vs baseline: 1.1463x; 1.1463x over previous
"""Trainium2 Bass kernel for nn_Mnist_lmdSplineKAN.

Sharding: 2D -- batch x4 (256 rows/core) by head-group x2 (5 heads = 320
out cols/core). All 8 cores do identical-shape work.

Math: the uniform-grid cubic B-spline basis is rewritten in the truncated
power basis,  f_j(z) = (1/6) sum_r (-1)^r C(4,r) (z+3-j-r)_+^3  with z=5x.
Splitting each (z-m)_+^3 into a smooth cubic (folded into the weights on
the host) plus a bounded one-sided cube leaves just 8 device feature
planes: d, d^2, d^3 (d = z-2.5), S1=(1-z)_+^3, S2=(2-z)_+^3, R3=(z-3)_+^3,
R4=(z-4)_+^3, and silu(x). The constant term becomes a bias row added via
a rank-1 ones-matmul. Features are fp16 stationary; weights fp16 moving;
PSUM fp32.

I=784 is tiled as 6 full chunks of 128 + 16 leftover rows; the leftover
rows x 8 planes pack into one K=128 matmul via an SBUF->SBUF repack.
Weights stream plane-major on the sync HWDGE queue in matmul consumption
order; warmup matmuls ramp the PE p-state before the real wavefront.
"""
import sys, types
import numpy as np

B, I, O, H = 1024, 784, 64, 10
NC, BG, OG = 8, 4, 2
BC = B // BG          # 256 batch rows per core
HOC = (H // OG) * O   # 320 output cols per core
D2C = (H // OG) * 32  # 160 hidden cols per core
NCH = 6               # full 128-row input chunks
PL = 16               # leftover input rows (chunk 6)
NP = 7                # feature planes: xc, xc^2, xc^3, R3, R4, S1, S2
NWARM = 16

# plane order: d, d2, silu, d3, R3, R4, S1, S2
C5 = np.array([1., -4., 6., -4., 1.]) / 6.0


def _tables():
    polyc = np.zeros((8, 4))
    tapS = np.zeros((8, 2))
    tapR = np.zeros((8, 2))
    for j in range(8):
        for r in range(5):
            m = j - 3 + r
            cc = C5[r]
            if m >= 5:
                continue
            if m in (3, 4):
                tapR[j, m - 3] += cc
            else:
                a = 2.5 - m
                polyc[j] += cc * np.array([a**3, 3 * a**2, 3 * a, 1.0])
                if m in (1, 2):
                    tapS[j, m - 1] += cc
    return polyc, tapS, tapR


def _install_ntff_hook():
    if "antenv.axon_hooks" in sys.modules:
        return
    try:
        import antenv
        mod = types.ModuleType("antenv.axon_hooks")
        _h = [None]
        mod.set_axon_ntff_profile_hook = lambda h: _h.__setitem__(0, h)
        mod.get_axon_ntff_profile_hook = lambda: _h[0]
        sys.modules["antenv.axon_hooks"] = mod
        antenv.axon_hooks = mod
        from trn_agent_boot.trn_boot import _ntff_profile_via_ctypes
        h = _ntff_profile_via_ctypes("/opt/axon/libaxon_pjrt.so")
        if h is not None:
            mod.set_axon_ntff_profile_hook(h)
    except Exception:
        pass


_CACHE = {}


def _build():
    if "nc" in _CACHE:
        return _CACHE["nc"]
    import concourse.bacc as bacc
    import concourse.bass as bass
    import concourse.tile as tile
    from concourse import mybir
    from contextlib import ExitStack

    f32, f16 = mybir.dt.float32, mybir.dt.float16
    ALU = mybir.AluOpType
    AF = mybir.ActivationFunctionType

    nc = bacc.Bacc("TRN2", target_bir_lowering=False, debug=False)
    x_d = nc.dram_tensor("x", (128, 7 * BC), f16, kind="ExternalInput").ap()
    WROW = NCH * HOC                       # 1920 elems per partition per plane
    w_d = nc.dram_tensor("w", (NP * 128 * WROW + NP * PL * HOC,), f16,
                         kind="ExternalInput").ap()
    b_d = nc.dram_tensor("brow", (1, HOC + D2C), f16, kind="ExternalInput").ap()
    cf16_d = nc.dram_tensor("cf16", (128, 3 * D2C + 128), f16,
                            kind="ExternalInput").ap()
    cf32_d = nc.dram_tensor("cf32", (128, D2C + 5), f32,
                            kind="ExternalInput").ap()
    out_d = nc.dram_tensor("out", (BC, 5), f32, kind="ExternalOutput").ap()

    with tile.TileContext(nc) as tc, ExitStack() as ctx:
        sb = ctx.enter_context(tc.tile_pool(name="sb", bufs=1))
        ps = ctx.enter_context(tc.tile_pool(name="ps", bufs=1, space="PSUM"))

        # ---- DMAs: sync HWDGE = bias row + x (features depend on x);
        #      gpsimd SWDGE = weight planes in consumption order, then
        #      packed chunk-6 and tail consts ----
        brow = sb.tile([1, HOC + D2C], f16, tag="brow")
        nc.sync.dma_start(brow[:], b_d)
        # x split: first 4 chunks on the (early-starting) sync HWDGE queue,
        # last 3 on SWDGE ahead of the weights; flat APs = big descriptors
        xt = sb.tile([128, 7, BC], f16, tag="xt")
        XSPL = 4 * BC
        nc.sync.dma_start(xt[:].rearrange("p c b -> p (c b)")[:, 0:XSPL],
                          x_d[:, 0:XSPL])
        nc.gpsimd.dma_start(xt[:].rearrange("p c b -> p (c b)")[:, XSPL:],
                            x_d[:, XSPL:])
        wAll = sb.tile([128, NP, NCH, HOC], f16, tag="wAll")
        for p in range(NP):
            src = bass.AP(tensor=w_d.tensor, offset=p * 128 * WROW,
                          ap=[[WROW, 128], [1, WROW]])
            nc.gpsimd.dma_start(
                wAll[:, p].rearrange("p c o -> p (c o)"), src)
        w6t = sb.tile([NP * PL, HOC], f16, tag="w6t")
        src6 = bass.AP(tensor=w_d.tensor, offset=NP * 128 * WROW,
                       ap=[[HOC, NP * PL], [1, HOC]])
        nc.gpsimd.dma_start(w6t[:], src6)

        cf16 = sb.tile([128, 3 * D2C + 128], f16, tag="cf16")
        nc.sync.dma_start(cf16[:], cf16_d)
        w1p = cf16[:, 0:3 * D2C].rearrange("p (k d) -> p k d", d=D2C)
        idt = cf16[:, 3 * D2C:]
        cf32 = sb.tile([128, D2C + 5], f32, tag="cf32")
        nc.sync.dma_start(cf32[:], cf32_d)
        w2b = cf32[:, 0:D2C]
        b2b = cf32[:, D2C:]

        ones = sb.tile([1, HOC], f16, tag="ones")
        nc.vector.memset(ones[:], 1.0)

        # force ACT tables to load during the DMA-wait window
        tl = sb.tile([1, 4], f16, tag="tl")
        for fn in (AF.Square, AF.Relu, AF.Tanh):
            nc.scalar.activation(tl[0:1, 0:1], ones[0:1, 0:1], fn)

        # ---- feature planes; xc = x - 0.5 comes pre-centered from host ----
        # plane order: 0:xc 1:xc^2 2:xc^3 3:R3 4:R4 5:S1 6:S2 where
        # R3=(x-0.6)+^3, R4=(x-0.8)+^3, S1=(0.2-x)+^3, S2=(0.4-x)+^3
        fall = sb.tile([128, NP - 1, 7, BC], f16, tag="fall")
        x2 = xt[:].rearrange("p c b -> p (c b)")

        def pl(p):
            if p == 0:
                return x2
            return fall[:, p - 1].rearrange("p c b -> p (c b)")

        def plc(p, c, bt):
            if p == 0:
                return xt[:, c, bt * 128:(bt + 1) * 128]
            return fall[:, p - 1, c, bt * 128:(bt + 1) * 128]

        def T(tag):
            return sb.tile([128, 7 * BC], f16, tag=tag, name=tag)

        bm3 = sb.tile([128, 1], f32, tag="bm3")
        nc.vector.memset(bm3[:], -0.3)
        bm1 = sb.tile([128, 1], f32, tag="bm1")
        nc.vector.memset(bm1[:], -0.1)
        s1 = T("s1"); s2 = T("s2"); r3 = T("r3"); r4 = T("r4")
        q1 = T("q1"); q2 = T("q2"); q3 = T("q3"); q4 = T("q4")
        # ACT: xc^2, s1=(0.2-x)+, s2=(0.4-x)+, s2^2
        nc.scalar.activation(pl(1), x2, AF.Square)
        nc.scalar.activation(s1[:], x2, AF.Relu, bias=bm3[:], scale=-1.0)
        nc.scalar.activation(s2[:], x2, AF.Relu, bias=bm1[:], scale=-1.0)
        nc.scalar.activation(q2[:], s2[:], AF.Square)
        # DVE: r3/r4 relus, xc^3, squares, cubes
        nc.vector.tensor_scalar(r3[:], x2, -0.1, 0.0, op0=ALU.add, op1=ALU.max)
        nc.vector.tensor_scalar(r4[:], x2, -0.3, 0.0, op0=ALU.add, op1=ALU.max)
        nc.vector.tensor_tensor(pl(2), pl(1), x2, op=ALU.mult)
        nc.vector.tensor_tensor(q3[:], r3[:], r3[:], op=ALU.mult)
        nc.vector.tensor_tensor(pl(3), q3[:], r3[:], op=ALU.mult)
        nc.vector.tensor_tensor(q4[:], r4[:], r4[:], op=ALU.mult)
        nc.vector.tensor_tensor(pl(4), q4[:], r4[:], op=ALU.mult)
        nc.vector.tensor_tensor(q1[:], s1[:], s1[:], op=ALU.mult)
        nc.vector.tensor_tensor(pl(5), q1[:], s1[:], op=ALU.mult)
        nc.vector.tensor_tensor(pl(6), q2[:], s2[:], op=ALU.mult)

        # ---- chunk-6 pack: 16 rows x 7 planes -> one K=112 tile ----
        f6 = sb.tile([NP * PL, BC], f16, tag="f6")
        nc.gpsimd.dma_start(f6[0:PL, :], xt[0:PL, 6, :])
        for p in range(1, NP):
            nc.gpsimd.dma_start(f6[p * PL:(p + 1) * PL, :],
                                fall[0:PL, p - 1, 6, :])

        # ---- matmuls ----
        # warmups depend only on the ones-memset, so the PE starts its
        # p-state ramp as soon as the engines come up (~7us before the
        # first weight plane lands)
        wu = ps.tile([128, HOC], f32, tag="wu")
        for k in range(NWARM):
            nc.tensor.matmul(wu[:], ones[0:1, 0:128], ones[:],
                             start=True, stop=True)

        y = [ps.tile([128, HOC], f32, tag=f"y{bt}", name=f"y{bt}")
             for bt in range(2)]
        for bt in range(2):
            nc.tensor.matmul(y[bt][:], ones[0:1, 0:128], brow[0:1, 0:HOC],
                             start=True, stop=False)
        for p in range(NP - 1):
            for c in range(NCH):
                for bt in range(2):
                    nc.tensor.matmul(y[bt][:], plc(p, c, bt),
                                     wAll[:, p, c, :], start=False, stop=False)
        # last plane + packed chunk-6: all of bt0 first so its PSUM bank
        # closes early and the bt0 tail overlaps bt1's matmuls
        for bt in range(2):
            for c in range(NCH):
                nc.tensor.matmul(y[bt][:], plc(NP - 1, c, bt),
                                 wAll[:, NP - 1, c, :], start=False, stop=False)
            nc.tensor.matmul(y[bt][:], f6[:, bt * 128:(bt + 1) * 128],
                             w6t[:], start=False, stop=True)

        # ---- tail per batch-tile: tanh, transpose, blockdiag MLP ----
        lgs = sb.tile([128, 2, 5], f32, tag="lgs")
        for bt in range(2):
            h1 = sb.tile([128, HOC], f16, tag=f"h1{bt}", name=f"h1{bt}")
            nc.scalar.activation(h1[:], y[bt][:], AF.Tanh)
            sts = []
            for k in range(3):
                kk = 128 if k < 2 else 64
                pt = ps.tile([128, 128], f16, tag=f"pt{k}",
                             name=f"pt{bt}{k}")
                nc.tensor.transpose(pt[0:kk, :], h1[:, k * 128:k * 128 + kk],
                                    idt)
                st = sb.tile([128, 128], f16, tag=f"st{bt}{k}",
                             name=f"st{bt}{k}")
                nc.vector.tensor_copy(st[0:kk, :], pt[0:kk, :])
                sts.append(st)
            ps2 = ps.tile([128, D2C], f32, tag=f"ps2{bt}", name=f"ps2{bt}")
            nc.tensor.matmul(ps2[:], ones[0:1, 0:128], brow[0:1, HOC:],
                             start=True, stop=False)
            for k in range(3):
                kk = 128 if k < 2 else 64
                nc.tensor.matmul(ps2[:], sts[k][0:kk, :], w1p[0:kk, k, :],
                                 start=False, stop=(k == 2))
            h2 = sb.tile([128, D2C], f32, tag=f"h2{bt}", name=f"h2{bt}")
            nc.scalar.activation(h2[:], ps2[:], AF.Tanh)
            prod = sb.tile([128, D2C], f32, tag=f"prod{bt}", name=f"prod{bt}")
            nc.vector.tensor_tensor(prod[:], h2[:], w2b, op=ALU.mult)
            red = sb.tile([128, 5], f32, tag=f"red{bt}", name=f"red{bt}")
            nc.vector.tensor_reduce(
                red[:], prod[:].rearrange("p (h d) -> p h d", d=32),
                axis=mybir.AxisListType.X, op=ALU.add)
            nc.vector.tensor_tensor(lgs[:, bt, :], red[:], b2b, op=ALU.add)
        # single out DMA: src (p, bt, col) -> dram row bt*128+p
        dst = bass.AP(tensor=out_d.tensor, offset=0,
                      ap=[[5, 128], [128 * 5, 2], [1, 5]])
        nc.sync.dma_start(dst, lgs[:])

    nc.compile()
    _CACHE["nc"] = nc
    return nc


def _prep_inputs(x, coef, scale_base, scale_sp, lmd, W1, b1, W2, b2):
    polyc, tapS, tapR = _tables()
    xf = np.asarray(x, np.float32).reshape(B, I)

    coef = np.asarray(coef, np.float64)
    eff = coef * np.asarray(scale_sp, np.float64)[..., None] \
        * np.asarray(lmd, np.float64)[:, :, None, None]        # (H, I, O, 8)
    W = eff.transpose(1, 3, 0, 2).reshape(I, 8, H * O)         # (I, 8, 640)
    sbl = (np.asarray(scale_base, np.float64)
           * np.asarray(lmd, np.float64)[:, :, None]
           ).transpose(1, 0, 2).reshape(I, H * O)

    # silu(x) lies (to ~1e-6) in the span of the 8-fn spline basis: fit it
    # and fold sbl * beta into the plane weights -- no silu plane on device
    g = np.linspace(0.0, 1.0, 4097)[:-1]
    gc = g - 0.5
    phi = np.stack([np.ones_like(g), gc, gc**2, gc**3,
                    np.maximum(g - 0.6, 0)**3, np.maximum(g - 0.8, 0)**3,
                    np.maximum(0.2 - g, 0)**3, np.maximum(0.4 - g, 0)**3], 1)
    beta = np.linalg.lstsq(phi, g / (1 + np.exp(-g)), rcond=None)[0]

    # fold: device plane order xc, xc^2, xc^3, R3', R4', S1', S2'
    # (d = 5*xc, so d-basis folds scale by 5^s; cubes by 125)
    Wp = np.empty((I, NP, H * O))
    Wp[:, 0] = 5.0 * np.einsum('j,ijo->io', polyc[:, 1], W) + beta[1] * sbl
    Wp[:, 1] = 25.0 * np.einsum('j,ijo->io', polyc[:, 2], W) + beta[2] * sbl
    Wp[:, 2] = 125.0 * np.einsum('j,ijo->io', polyc[:, 3], W) + beta[3] * sbl
    Wp[:, 3] = 125.0 * np.einsum('j,ijo->io', tapR[:, 0], W) + beta[4] * sbl
    Wp[:, 4] = 125.0 * np.einsum('j,ijo->io', tapR[:, 1], W) + beta[5] * sbl
    Wp[:, 5] = 125.0 * np.einsum('j,ijo->io', tapS[:, 0], W) + beta[6] * sbl
    Wp[:, 6] = 125.0 * np.einsum('j,ijo->io', tapS[:, 1], W) + beta[7] * sbl
    bias_full = np.einsum('j,ijo->o', polyc[:, 0], W) \
        + beta[0] * sbl.sum(0)                                 # (640,)

    W1 = np.asarray(W1, np.float64)
    W2 = np.asarray(W2, np.float64).reshape(H * 32)
    b1 = np.asarray(b1, np.float64).reshape(H * 32)
    b2 = np.asarray(b2, np.float64).reshape(H)

    per_og = []
    for og in range(OG):
        hs = slice(og * HOC, (og + 1) * HOC)
        # weight stream: 8 plane pieces [128, 6*320] then packed chunk-6
        pieces = []
        for p in range(NP):
            blk = Wp[0:NCH * 128, p, hs].reshape(NCH, 128, HOC)
            pieces.append(np.ascontiguousarray(
                blk.transpose(1, 0, 2)).reshape(-1))
        w6 = np.zeros((NP * PL, HOC))
        for p in range(NP):
            w6[p * PL:(p + 1) * PL] = Wp[NCH * 128:I, p, hs]
        pieces.append(np.ascontiguousarray(w6).reshape(-1))
        wdev = np.concatenate(pieces).astype(np.float16)

        brow = np.zeros((1, HOC + D2C))
        brow[0, 0:HOC] = bias_full[hs]
        brow[0, HOC:] = b1[og * D2C:(og + 1) * D2C]
        brow = brow.astype(np.float16)

        w1bd = np.zeros((HOC, D2C))
        for hl in range(H // OG):
            w1bd[hl * O:(hl + 1) * O, hl * 32:(hl + 1) * 32] = W1[og * (H // OG) + hl]
        w1dev = np.zeros((128, 3, D2C))
        w1dev[:, 0] = w1bd[0:128]
        w1dev[:, 1] = w1bd[128:256]
        w1dev[0:64, 2] = w1bd[256:HOC]
        cf16 = np.concatenate([w1dev.reshape(128, 3 * D2C),
                               np.eye(128)], 1).astype(np.float16)
        cf32 = np.concatenate([
            np.broadcast_to(W2[og * D2C:(og + 1) * D2C], (128, D2C)),
            np.broadcast_to(b2[og * 5:(og + 1) * 5], (128, 5))],
            1).astype(np.float32)
        per_og.append((wdev, brow, cf16, cf32))

    in_maps = []
    for core in range(NC):
        bg, og = core % BG, core // BG
        xs = (xf[bg * BC:(bg + 1) * BC].T - 0.5).astype(np.float16)  # (784, 256)
        xdev = np.zeros((7, 128, BC), np.float16)
        xdev.reshape(7 * 128, BC)[0:I] = xs
        xdev = np.ascontiguousarray(xdev.transpose(1, 0, 2)).reshape(128, 7 * BC)
        wdev, brow, cf16, cf32 = per_og[og]
        in_maps.append({"x": xdev, "w": wdev, "brow": brow,
                        "cf16": cf16, "cf32": cf32})
    return in_maps


def run(inputs, trace=False, tmpdir=None):
    _install_ntff_hook()
    from concourse.bass_utils import run_bass_kernel_spmd
    nc = _build()
    in_maps = _prep_inputs(**inputs)
    res = run_bass_kernel_spmd(nc, in_maps, core_ids=list(range(NC)),
                               trace=trace, tmpdir=tmpdir)
    out = np.empty((B, H), np.float32)
    for core in range(NC):
        bg, og = core % BG, core // BG
        out[bg * BC:(bg + 1) * BC, og * 5:(og + 1) * 5] = res.results[core]["out"]
    return out, res


def kernel(**inputs):
    out, _ = run(inputs)
    return out


# revision 39
# speedup vs baseline: 1.1819x; 1.0310x over previous
"""Trainium2 Bass kernel for nn_Mnist_lmdSplineKAN.

Sharding: 2D -- batch x4 (256 rows/core) by head-group x2 (5 heads = 320
out cols/core). All 8 cores do identical-shape work.

Math: the uniform-grid cubic B-spline basis is rewritten in the truncated
power basis,  f_j(z) = (1/6) sum_r (-1)^r C(4,r) (z+3-j-r)_+^3,  z = 5x.
Each (z-m)_+^3 splits into a smooth cubic (folded into the weights on the
host) plus a bounded one-sided cube, and silu(x) itself lies in the span
of this spline basis (lstsq fit, ~1e-6 residual) so it folds in too.
That leaves just 7 device feature planes: xc, xc^2, xc^3 (xc = x - 0.5,
pre-centered on host), R3=(x-.6)+^3, R4=(x-.8)+^3, S1=(.2-x)+^3,
S2=(.4-x)+^3; the constant term becomes a bias row added via a rank-1
ones-matmul. Features fp16 stationary, weights fp16 moving, PSUM fp32.

I=784 is tiled as 6 full chunks of 128 + 16 leftover rows; the leftover
rows x 7 planes pack into one K=112 matmul via SBUF->SBUF repack DMAs.
Weights stream plane-major on the SWDGE queue (~330 B/ns; the HWDGE
queues only sustain ~100-170) in matmul consumption order, x rides both
queues, and ones-matmul warmups keep the PE busy through its DVFS ramp.
The last plane's matmuls close batch-tile 0's PSUM bank first so its
tanh/transpose/MLP tail overlaps batch-tile 1's matmuls.
"""
import sys, types
import numpy as np

B, I, O, H = 1024, 784, 64, 10
NC, BG, OG = 8, 4, 2
BC = B // BG          # 256 batch rows per core
HOC = (H // OG) * O   # 320 output cols per core
D2C = (H // OG) * 32  # 160 hidden cols per core
NCH = 6               # full 128-row input chunks
PL = 16               # leftover input rows (chunk 6)
NP = 7                # feature planes: xc, xc^2, xc^3, R3, R4, S1, S2
NWARM = 16

C5 = np.array([1., -4., 6., -4., 1.]) / 6.0


def _tables():
    polyc = np.zeros((8, 4))
    tapS = np.zeros((8, 2))
    tapR = np.zeros((8, 2))
    for j in range(8):
        for r in range(5):
            m = j - 3 + r
            cc = C5[r]
            if m >= 5:
                continue
            if m in (3, 4):
                tapR[j, m - 3] += cc
            else:
                a = 2.5 - m
                polyc[j] += cc * np.array([a**3, 3 * a**2, 3 * a, 1.0])
                if m in (1, 2):
                    tapS[j, m - 1] += cc
    return polyc, tapS, tapR


def _install_ntff_hook():
    if "antenv.axon_hooks" in sys.modules:
        return
    try:
        import antenv
        mod = types.ModuleType("antenv.axon_hooks")
        _h = [None]
        mod.set_axon_ntff_profile_hook = lambda h: _h.__setitem__(0, h)
        mod.get_axon_ntff_profile_hook = lambda: _h[0]
        sys.modules["antenv.axon_hooks"] = mod
        antenv.axon_hooks = mod
        from trn_agent_boot.trn_boot import _ntff_profile_via_ctypes
        h = _ntff_profile_via_ctypes("/opt/axon/libaxon_pjrt.so")
        if h is not None:
            mod.set_axon_ntff_profile_hook(h)
    except Exception:
        pass


_CACHE = {}


def _build():
    if "nc" in _CACHE:
        return _CACHE["nc"]
    import concourse.bacc as bacc
    import concourse.bass as bass
    import concourse.tile as tile
    from concourse import mybir
    from contextlib import ExitStack

    f32, f16 = mybir.dt.float32, mybir.dt.float16
    ALU = mybir.AluOpType
    AF = mybir.ActivationFunctionType

    nc = bacc.Bacc("TRN2", target_bir_lowering=False, debug=False)
    x_d = nc.dram_tensor("x", (128, 7 * BC), f16, kind="ExternalInput").ap()
    WROW = NCH * HOC                       # 1920 elems per partition per plane
    w_d = nc.dram_tensor("w", (NP * 128 * WROW + NP * PL * HOC,), f16,
                         kind="ExternalInput").ap()
    b_d = nc.dram_tensor("brow", (1, HOC + D2C), f16, kind="ExternalInput").ap()
    cf16_d = nc.dram_tensor("cf16", (128, 3 * D2C + 128), f16,
                            kind="ExternalInput").ap()
    cf32_d = nc.dram_tensor("cf32", (128, D2C + 5), f32,
                            kind="ExternalInput").ap()
    out_d = nc.dram_tensor("out", (BC, 5), f32, kind="ExternalOutput").ap()

    with tile.TileContext(nc) as tc, ExitStack() as ctx:
        sb = ctx.enter_context(tc.tile_pool(name="sb", bufs=1))
        ps = ctx.enter_context(tc.tile_pool(name="ps", bufs=1, space="PSUM"))

        # ---- DMAs: sync HWDGE = bias row + x (features depend on x);
        #      gpsimd SWDGE = weight planes in consumption order, then
        #      packed chunk-6 and tail consts ----
        brow = sb.tile([1, HOC + D2C], f16, tag="brow")
        nc.sync.dma_start(brow[:], b_d)
        # x split: first 4 chunks on the (early-starting) sync HWDGE queue,
        # last 3 on SWDGE ahead of the weights; flat APs = big descriptors
        xt = sb.tile([128, 7, BC], f16, tag="xt")
        XSPL = 4 * BC
        nc.sync.dma_start(xt[:].rearrange("p c b -> p (c b)")[:, 0:XSPL],
                          x_d[:, 0:XSPL])
        nc.gpsimd.dma_start(xt[:].rearrange("p c b -> p (c b)")[:, XSPL:],
                            x_d[:, XSPL:])
        wAll = sb.tile([128, NP, NCH, HOC], f16, tag="wAll")
        for p in range(NP):
            src = bass.AP(tensor=w_d.tensor, offset=p * 128 * WROW,
                          ap=[[WROW, 128], [1, WROW]])
            nc.gpsimd.dma_start(
                wAll[:, p].rearrange("p c o -> p (c o)"), src)
        w6t = sb.tile([NP * PL, HOC], f16, tag="w6t")
        src6 = bass.AP(tensor=w_d.tensor, offset=NP * 128 * WROW,
                       ap=[[HOC, NP * PL], [1, HOC]])
        nc.gpsimd.dma_start(w6t[:], src6)

        cf16 = sb.tile([128, 3 * D2C + 128], f16, tag="cf16")
        nc.sync.dma_start(cf16[:], cf16_d)
        w1p = cf16[:, 0:3 * D2C].rearrange("p (k d) -> p k d", d=D2C)
        idt = cf16[:, 3 * D2C:]
        cf32 = sb.tile([128, D2C + 5], f32, tag="cf32")
        nc.sync.dma_start(cf32[:], cf32_d)
        w2b = cf32[:, 0:D2C]
        b2b = cf32[:, D2C:]

        ones = sb.tile([1, HOC], f16, tag="ones")
        nc.vector.memset(ones[:], 1.0)

        # force ACT tables to load during the DMA-wait window
        tl = sb.tile([1, 4], f16, tag="tl")
        for fn in (AF.Square, AF.Relu, AF.Tanh):
            nc.scalar.activation(tl[0:1, 0:1], ones[0:1, 0:1], fn)

        # ---- feature planes; xc = x - 0.5 comes pre-centered from host ----
        # plane order: 0:xc 1:xc^2 2:xc^3 3:R3 4:R4 5:S1 6:S2 where
        # R3=(x-0.6)+^3, R4=(x-0.8)+^3, S1=(0.2-x)+^3, S2=(0.4-x)+^3
        fall = sb.tile([128, NP - 1, 7, BC], f16, tag="fall")
        x2 = xt[:].rearrange("p c b -> p (c b)")

        def pl(p):
            if p == 0:
                return x2
            return fall[:, p - 1].rearrange("p c b -> p (c b)")

        def plc(p, c, bt):
            if p == 0:
                return xt[:, c, bt * 128:(bt + 1) * 128]
            return fall[:, p - 1, c, bt * 128:(bt + 1) * 128]

        def T(tag):
            return sb.tile([128, 7 * BC], f16, tag=tag, name=tag)

        bm3 = sb.tile([128, 1], f32, tag="bm3")
        nc.vector.memset(bm3[:], -0.3)
        bm1 = sb.tile([128, 1], f32, tag="bm1")
        nc.vector.memset(bm1[:], -0.1)
        s1 = T("s1"); s2 = T("s2"); r3 = T("r3"); r4 = T("r4")
        q1 = T("q1"); q2 = T("q2"); q3 = T("q3"); q4 = T("q4")
        # ACT: xc^2, s1=(0.2-x)+, s2=(0.4-x)+, s2^2
        nc.scalar.activation(pl(1), x2, AF.Square)
        nc.scalar.activation(s1[:], x2, AF.Relu, bias=bm3[:], scale=-1.0)
        nc.scalar.activation(s2[:], x2, AF.Relu, bias=bm1[:], scale=-1.0)
        nc.scalar.activation(q2[:], s2[:], AF.Square)
        # DVE: r3/r4 relus, xc^3, squares, cubes
        nc.vector.tensor_scalar(r3[:], x2, -0.1, 0.0, op0=ALU.add, op1=ALU.max)
        nc.vector.tensor_scalar(r4[:], x2, -0.3, 0.0, op0=ALU.add, op1=ALU.max)
        nc.vector.tensor_tensor(pl(2), pl(1), x2, op=ALU.mult)
        nc.vector.tensor_tensor(q3[:], r3[:], r3[:], op=ALU.mult)
        nc.vector.tensor_tensor(pl(3), q3[:], r3[:], op=ALU.mult)
        nc.vector.tensor_tensor(q4[:], r4[:], r4[:], op=ALU.mult)
        nc.vector.tensor_tensor(pl(4), q4[:], r4[:], op=ALU.mult)
        nc.vector.tensor_tensor(q1[:], s1[:], s1[:], op=ALU.mult)
        nc.vector.tensor_tensor(pl(5), q1[:], s1[:], op=ALU.mult)
        nc.vector.tensor_tensor(pl(6), q2[:], s2[:], op=ALU.mult)

        # ---- chunk-6 pack: 16 rows x 7 planes -> one K=112 tile ----
        f6 = sb.tile([NP * PL, BC], f16, tag="f6")
        nc.gpsimd.dma_start(f6[0:PL, :], xt[0:PL, 6, :])
        for p in range(1, NP):
            nc.gpsimd.dma_start(f6[p * PL:(p + 1) * PL, :],
                                fall[0:PL, p - 1, 6, :])

        # ---- matmuls ----
        # warmups depend only on the ones-memset, so the PE starts its
        # p-state ramp as soon as the engines come up (~7us before the
        # first weight plane lands)
        wu = ps.tile([128, HOC], f32, tag="wu")
        for k in range(NWARM):
            nc.tensor.matmul(wu[:], ones[0:1, 0:128], ones[:],
                             start=True, stop=True)

        y = [ps.tile([128, HOC], f32, tag=f"y{bt}", name=f"y{bt}")
             for bt in range(2)]
        for bt in range(2):
            nc.tensor.matmul(y[bt][:], ones[0:1, 0:128], brow[0:1, 0:HOC],
                             start=True, stop=False)
        for p in range(NP - 1):
            for c in range(NCH):
                for bt in range(2):
                    nc.tensor.matmul(y[bt][:], plc(p, c, bt),
                                     wAll[:, p, c, :], start=False, stop=False)
        # last plane + packed chunk-6: all of bt0 first so its PSUM bank
        # closes early and the bt0 tail overlaps bt1's matmuls
        for bt in range(2):
            for c in range(NCH):
                nc.tensor.matmul(y[bt][:], plc(NP - 1, c, bt),
                                 wAll[:, NP - 1, c, :], start=False, stop=False)
            nc.tensor.matmul(y[bt][:], f6[:, bt * 128:(bt + 1) * 128],
                             w6t[:], start=False, stop=True)

        # ---- tail per batch-tile: tanh, transpose, blockdiag MLP ----
        lgs = sb.tile([128, 2, 5], f32, tag="lgs")
        for bt in range(2):
            h1 = sb.tile([128, HOC], f16, tag=f"h1{bt}", name=f"h1{bt}")
            nc.scalar.activation(h1[:], y[bt][:], AF.Tanh)
            sts = []
            for k in range(3):
                kk = 128 if k < 2 else 64
                pt = ps.tile([128, 128], f16, tag=f"pt{k}",
                             name=f"pt{bt}{k}")
                nc.tensor.transpose(pt[0:kk, :], h1[:, k * 128:k * 128 + kk],
                                    idt)
                st = sb.tile([128, 128], f16, tag=f"st{bt}{k}",
                             name=f"st{bt}{k}")
                nc.vector.tensor_copy(st[0:kk, :], pt[0:kk, :])
                sts.append(st)
            ps2 = ps.tile([128, D2C], f32, tag=f"ps2{bt}", name=f"ps2{bt}")
            nc.tensor.matmul(ps2[:], ones[0:1, 0:128], brow[0:1, HOC:],
                             start=True, stop=False)
            for k in range(3):
                kk = 128 if k < 2 else 64
                nc.tensor.matmul(ps2[:], sts[k][0:kk, :], w1p[0:kk, k, :],
                                 start=False, stop=(k == 2))
            h2 = sb.tile([128, D2C], f32, tag=f"h2{bt}", name=f"h2{bt}")
            nc.scalar.activation(h2[:], ps2[:], AF.Tanh)
            prod = sb.tile([128, D2C], f32, tag=f"prod{bt}", name=f"prod{bt}")
            nc.vector.tensor_tensor(prod[:], h2[:], w2b, op=ALU.mult)
            red = sb.tile([128, 5], f32, tag=f"red{bt}", name=f"red{bt}")
            nc.vector.tensor_reduce(
                red[:], prod[:].rearrange("p (h d) -> p h d", d=32),
                axis=mybir.AxisListType.X, op=ALU.add)
            nc.vector.tensor_tensor(lgs[:, bt, :], red[:], b2b, op=ALU.add)
        # single out DMA: src (p, bt, col) -> dram row bt*128+p
        dst = bass.AP(tensor=out_d.tensor, offset=0,
                      ap=[[5, 128], [128 * 5, 2], [1, 5]])
        nc.sync.dma_start(dst, lgs[:])

    nc.compile()
    _CACHE["nc"] = nc
    return nc


def _prep_inputs(x, coef, scale_base, scale_sp, lmd, W1, b1, W2, b2):
    polyc, tapS, tapR = _tables()
    xf = np.asarray(x, np.float32).reshape(B, I)

    coef = np.asarray(coef, np.float64)
    eff = coef * np.asarray(scale_sp, np.float64)[..., None] \
        * np.asarray(lmd, np.float64)[:, :, None, None]        # (H, I, O, 8)
    W = eff.transpose(1, 3, 0, 2).reshape(I, 8, H * O)         # (I, 8, 640)
    sbl = (np.asarray(scale_base, np.float64)
           * np.asarray(lmd, np.float64)[:, :, None]
           ).transpose(1, 0, 2).reshape(I, H * O)

    # silu(x) lies (to ~1e-6) in the span of the 8-fn spline basis: fit it
    # and fold sbl * beta into the plane weights -- no silu plane on device
    g = np.linspace(0.0, 1.0, 4097)[:-1]
    gc = g - 0.5
    phi = np.stack([np.ones_like(g), gc, gc**2, gc**3,
                    np.maximum(g - 0.6, 0)**3, np.maximum(g - 0.8, 0)**3,
                    np.maximum(0.2 - g, 0)**3, np.maximum(0.4 - g, 0)**3], 1)
    beta = np.linalg.lstsq(phi, g / (1 + np.exp(-g)), rcond=None)[0]

    # fold: device plane order xc, xc^2, xc^3, R3', R4', S1', S2'
    # (d = 5*xc, so d-basis folds scale by 5^s; cubes by 125)
    Wp = np.empty((I, NP, H * O))
    Wp[:, 0] = 5.0 * np.einsum('j,ijo->io', polyc[:, 1], W) + beta[1] * sbl
    Wp[:, 1] = 25.0 * np.einsum('j,ijo->io', polyc[:, 2], W) + beta[2] * sbl
    Wp[:, 2] = 125.0 * np.einsum('j,ijo->io', polyc[:, 3], W) + beta[3] * sbl
    Wp[:, 3] = 125.0 * np.einsum('j,ijo->io', tapR[:, 0], W) + beta[4] * sbl
    Wp[:, 4] = 125.0 * np.einsum('j,ijo->io', tapR[:, 1], W) + beta[5] * sbl
    Wp[:, 5] = 125.0 * np.einsum('j,ijo->io', tapS[:, 0], W) + beta[6] * sbl
    Wp[:, 6] = 125.0 * np.einsum('j,ijo->io', tapS[:, 1], W) + beta[7] * sbl
    bias_full = np.einsum('j,ijo->o', polyc[:, 0], W) \
        + beta[0] * sbl.sum(0)                                 # (640,)

    W1 = np.asarray(W1, np.float64)
    W2 = np.asarray(W2, np.float64).reshape(H * 32)
    b1 = np.asarray(b1, np.float64).reshape(H * 32)
    b2 = np.asarray(b2, np.float64).reshape(H)

    per_og = []
    for og in range(OG):
        hs = slice(og * HOC, (og + 1) * HOC)
        # weight stream: 8 plane pieces [128, 6*320] then packed chunk-6
        pieces = []
        for p in range(NP):
            blk = Wp[0:NCH * 128, p, hs].reshape(NCH, 128, HOC)
            pieces.append(np.ascontiguousarray(
                blk.transpose(1, 0, 2)).reshape(-1))
        w6 = np.zeros((NP * PL, HOC))
        for p in range(NP):
            w6[p * PL:(p + 1) * PL] = Wp[NCH * 128:I, p, hs]
        pieces.append(np.ascontiguousarray(w6).reshape(-1))
        wdev = np.concatenate(pieces).astype(np.float16)

        brow = np.zeros((1, HOC + D2C))
        brow[0, 0:HOC] = bias_full[hs]
        brow[0, HOC:] = b1[og * D2C:(og + 1) * D2C]
        brow = brow.astype(np.float16)

        w1bd = np.zeros((HOC, D2C))
        for hl in range(H // OG):
            w1bd[hl * O:(hl + 1) * O, hl * 32:(hl + 1) * 32] = W1[og * (H // OG) + hl]
        w1dev = np.zeros((128, 3, D2C))
        w1dev[:, 0] = w1bd[0:128]
        w1dev[:, 1] = w1bd[128:256]
        w1dev[0:64, 2] = w1bd[256:HOC]
        cf16 = np.concatenate([w1dev.reshape(128, 3 * D2C),
                               np.eye(128)], 1).astype(np.float16)
        cf32 = np.concatenate([
            np.broadcast_to(W2[og * D2C:(og + 1) * D2C], (128, D2C)),
            np.broadcast_to(b2[og * 5:(og + 1) * 5], (128, 5))],
            1).astype(np.float32)
        per_og.append((wdev, brow, cf16, cf32))

    in_maps = []
    for core in range(NC):
        bg, og = core % BG, core // BG
        xs = (xf[bg * BC:(bg + 1) * BC].T - 0.5).astype(np.float16)  # (784, 256)
        xdev = np.zeros((7, 128, BC), np.float16)
        xdev.reshape(7 * 128, BC)[0:I] = xs
        xdev = np.ascontiguousarray(xdev.transpose(1, 0, 2)).reshape(128, 7 * BC)
        wdev, brow, cf16, cf32 = per_og[og]
        in_maps.append({"x": xdev, "w": wdev, "brow": brow,
                        "cf16": cf16, "cf32": cf32})
    return in_maps


def run(inputs, trace=False, tmpdir=None):
    _install_ntff_hook()
    from concourse.bass_utils import run_bass_kernel_spmd
    nc = _build()
    in_maps = _prep_inputs(**inputs)
    res = run_bass_kernel_spmd(nc, in_maps, core_ids=list(range(NC)),
                               trace=trace, tmpdir=tmpdir)
    out = np.empty((B, H), np.float32)
    for core in range(NC):
        bg, og = core % BG, core // BG
        out[bg * BC:(bg + 1) * BC, og * 5:(og + 1) * 5] = res.results[core]["out"]
    return out, res


def kernel(**inputs):
    out, _ = run(inputs)
    return out


# revision 40
# speedup vs baseline: 1.1959x; 1.0119x over previous
"""Trainium2 Bass kernel for nn_Mnist_lmdSplineKAN.

Sharding: 2D -- batch x4 (256 rows/core) by head-group x2 (5 heads = 320
out cols/core). All 8 cores do identical-shape work.

Math: the uniform-grid cubic B-spline basis is rewritten in the truncated
power basis,  f_j(z) = (1/6) sum_r (-1)^r C(4,r) (z+3-j-r)_+^3,  z = 5x.
Each (z-m)_+^3 splits into a smooth cubic (folded into the weights on the
host) plus a bounded one-sided cube, and silu(x) itself lies in the span
of this spline basis (lstsq fit, ~1e-6 residual) so it folds in too.
That leaves just 7 device feature planes: xc, xc^2, xc^3 (xc = x - 0.5,
pre-centered on host), R3=(x-.6)+^3, R4=(x-.8)+^3, S1=(.2-x)+^3,
S2=(.4-x)+^3; the constant term becomes a bias row added via a rank-1
ones-matmul. Features fp16 stationary, weights fp16 moving, PSUM fp32.

I=784 is tiled as 6 full chunks of 128 + 16 leftover rows; the leftover
rows x 7 planes pack into one K=112 matmul via SBUF->SBUF repack DMAs.
Weights stream plane-major on the SWDGE queue (~330 B/ns; the HWDGE
queues only sustain ~100-170) in matmul consumption order, x rides both
queues, and ones-matmul warmups keep the PE busy through its DVFS ramp.
The last plane's matmuls close batch-tile 0's PSUM bank first so its
tanh/transpose/MLP tail overlaps batch-tile 1's matmuls.
"""
import sys, types
import numpy as np

B, I, O, H = 1024, 784, 64, 10
NC, BG, OG = 8, 4, 2
BC = B // BG          # 256 batch rows per core
HOC = (H // OG) * O   # 320 output cols per core
D2C = (H // OG) * 32  # 160 hidden cols per core
NCH = 6               # full 128-row input chunks
PL = 16               # leftover input rows (chunk 6)
NP = 7                # feature planes: xc, xc^2, xc^3, R3, R4, S1, S2
NWARM = 16

C5 = np.array([1., -4., 6., -4., 1.]) / 6.0


def _tables():
    polyc = np.zeros((8, 4))
    tapS = np.zeros((8, 2))
    tapR = np.zeros((8, 2))
    for j in range(8):
        for r in range(5):
            m = j - 3 + r
            cc = C5[r]
            if m >= 5:
                continue
            if m in (3, 4):
                tapR[j, m - 3] += cc
            else:
                a = 2.5 - m
                polyc[j] += cc * np.array([a**3, 3 * a**2, 3 * a, 1.0])
                if m in (1, 2):
                    tapS[j, m - 1] += cc
    return polyc, tapS, tapR


def _install_ntff_hook():
    if "antenv.axon_hooks" in sys.modules:
        return
    try:
        import antenv
        mod = types.ModuleType("antenv.axon_hooks")
        _h = [None]
        mod.set_axon_ntff_profile_hook = lambda h: _h.__setitem__(0, h)
        mod.get_axon_ntff_profile_hook = lambda: _h[0]
        sys.modules["antenv.axon_hooks"] = mod
        antenv.axon_hooks = mod
        from trn_agent_boot.trn_boot import _ntff_profile_via_ctypes
        h = _ntff_profile_via_ctypes("/opt/axon/libaxon_pjrt.so")
        if h is not None:
            mod.set_axon_ntff_profile_hook(h)
    except Exception:
        pass


_CACHE = {}


def _build():
    if "nc" in _CACHE:
        return _CACHE["nc"]
    import concourse.bacc as bacc
    import concourse.bass as bass
    import concourse.tile as tile
    from concourse import mybir
    from contextlib import ExitStack

    f32, f16 = mybir.dt.float32, mybir.dt.float16
    ALU = mybir.AluOpType
    AF = mybir.ActivationFunctionType

    nc = bacc.Bacc("TRN2", target_bir_lowering=False, debug=False)
    x_d = nc.dram_tensor("x", (128, 7 * BC), f16, kind="ExternalInput").ap()
    WROW = NCH * HOC                       # 1920 elems per partition per plane
    w_d = nc.dram_tensor("w", (NP * 128 * WROW + NP * PL * HOC,), f16,
                         kind="ExternalInput").ap()
    b_d = nc.dram_tensor("brow", (1, HOC + D2C), f16, kind="ExternalInput").ap()
    cf16_d = nc.dram_tensor("cf16", (128, 3 * D2C + 128), f16,
                            kind="ExternalInput").ap()
    cf32_d = nc.dram_tensor("cf32", (128, D2C + 5), f32,
                            kind="ExternalInput").ap()
    out_d = nc.dram_tensor("out", (BC, 5), f32, kind="ExternalOutput").ap()

    with tile.TileContext(nc) as tc, ExitStack() as ctx:
        sb = ctx.enter_context(tc.tile_pool(name="sb", bufs=1))
        ps = ctx.enter_context(tc.tile_pool(name="ps", bufs=1, space="PSUM"))

        # ---- DMAs: sync HWDGE = bias row + x (features depend on x);
        #      gpsimd SWDGE = weight planes in consumption order, then
        #      packed chunk-6 and tail consts ----
        brow = sb.tile([1, HOC + D2C], f16, tag="brow")
        nc.sync.dma_start(brow[:], b_d)
        # x split: first 4 chunks on the (early-starting) sync HWDGE queue,
        # last 3 on SWDGE ahead of the weights; flat APs = big descriptors
        xt = sb.tile([128, 7, BC], f16, tag="xt")
        XSPL = 4 * BC
        nc.sync.dma_start(xt[:].rearrange("p c b -> p (c b)")[:, 0:XSPL],
                          x_d[:, 0:XSPL])
        nc.gpsimd.dma_start(xt[:].rearrange("p c b -> p (c b)")[:, XSPL:],
                            x_d[:, XSPL:])
        wAll = sb.tile([128, NP, NCH, HOC], f16, tag="wAll")
        for p in range(NP):
            src = bass.AP(tensor=w_d.tensor, offset=p * 128 * WROW,
                          ap=[[WROW, 128], [1, WROW]])
            nc.gpsimd.dma_start(
                wAll[:, p].rearrange("p c o -> p (c o)"), src)
        w6t = sb.tile([NP * PL, HOC], f16, tag="w6t")
        src6 = bass.AP(tensor=w_d.tensor, offset=NP * 128 * WROW,
                       ap=[[HOC, NP * PL], [1, HOC]])
        nc.gpsimd.dma_start(w6t[:], src6)

        cf16 = sb.tile([128, 3 * D2C + 128], f16, tag="cf16")
        nc.sync.dma_start(cf16[:], cf16_d)
        w1p = cf16[:, 0:3 * D2C].rearrange("p (k d) -> p k d", d=D2C)
        idt = cf16[:, 3 * D2C:]
        cf32 = sb.tile([128, D2C + 5], f32, tag="cf32")
        nc.sync.dma_start(cf32[:], cf32_d)
        w2b = cf32[:, 0:D2C]
        b2b = cf32[:, D2C:]

        ones = sb.tile([1, HOC], f16, tag="ones")
        nc.vector.memset(ones[:], 1.0)

        # force ACT tables to load during the DMA-wait window
        tl = sb.tile([1, 4], f16, tag="tl")
        for fn in (AF.Square, AF.Relu, AF.Tanh):
            nc.scalar.activation(tl[0:1, 0:1], ones[0:1, 0:1], fn)

        # ---- feature planes; xc = x - 0.5 comes pre-centered from host ----
        # plane order: 0:xc 1:xc^2 2:xc^3 3:R3 4:R4 5:S1 6:S2 where
        # R3=(x-0.6)+^3, R4=(x-0.8)+^3, S1=(0.2-x)+^3, S2=(0.4-x)+^3
        fall = sb.tile([128, NP - 1, 7, BC], f16, tag="fall")
        x2 = xt[:].rearrange("p c b -> p (c b)")

        def pl(p):
            if p == 0:
                return x2
            return fall[:, p - 1].rearrange("p c b -> p (c b)")

        def plc(p, c, bt):
            if p == 0:
                return xt[:, c, bt * 128:(bt + 1) * 128]
            return fall[:, p - 1, c, bt * 128:(bt + 1) * 128]

        def T(tag):
            return sb.tile([128, 7 * BC], f16, tag=tag, name=tag)

        bm3 = sb.tile([128, 1], f32, tag="bm3")
        nc.vector.memset(bm3[:], -0.3)
        bm1 = sb.tile([128, 1], f32, tag="bm1")
        nc.vector.memset(bm1[:], -0.1)
        s1 = T("s1"); s2 = T("s2"); r3 = T("r3"); r4 = T("r4")
        q1 = T("q1"); q2 = T("q2"); q3 = T("q3"); q4 = T("q4")
        # ACT: xc^2, s1=(0.2-x)+, s2=(0.4-x)+, s2^2
        nc.scalar.activation(pl(1), x2, AF.Square)
        nc.scalar.activation(s1[:], x2, AF.Relu, bias=bm3[:], scale=-1.0)
        nc.scalar.activation(s2[:], x2, AF.Relu, bias=bm1[:], scale=-1.0)
        nc.scalar.activation(q2[:], s2[:], AF.Square)
        # DVE: r3/r4 relus, xc^3, squares, cubes
        nc.vector.tensor_scalar(r3[:], x2, -0.1, 0.0, op0=ALU.add, op1=ALU.max)
        nc.vector.tensor_scalar(r4[:], x2, -0.3, 0.0, op0=ALU.add, op1=ALU.max)
        nc.vector.tensor_tensor(pl(2), pl(1), x2, op=ALU.mult)
        nc.vector.tensor_tensor(q3[:], r3[:], r3[:], op=ALU.mult)
        nc.vector.tensor_tensor(pl(3), q3[:], r3[:], op=ALU.mult)
        nc.vector.tensor_tensor(q4[:], r4[:], r4[:], op=ALU.mult)
        nc.vector.tensor_tensor(pl(4), q4[:], r4[:], op=ALU.mult)
        nc.vector.tensor_tensor(q1[:], s1[:], s1[:], op=ALU.mult)
        nc.vector.tensor_tensor(pl(5), q1[:], s1[:], op=ALU.mult)
        nc.vector.tensor_tensor(pl(6), q2[:], s2[:], op=ALU.mult)

        # ---- chunk-6 pack: 16 rows x 7 planes -> one K=112 tile ----
        f6 = sb.tile([NP * PL, BC], f16, tag="f6")
        nc.sync.dma_start(f6[0:PL, :], xt[0:PL, 6, :])
        for p in range(1, NP):
            nc.sync.dma_start(f6[p * PL:(p + 1) * PL, :],
                              fall[0:PL, p - 1, 6, :])

        # ---- matmuls ----
        # warmups depend only on the ones-memset, so the PE starts its
        # p-state ramp as soon as the engines come up (~7us before the
        # first weight plane lands)
        wu = ps.tile([128, HOC], f32, tag="wu")
        for k in range(NWARM):
            nc.tensor.matmul(wu[:], ones[0:1, 0:128], ones[:],
                             start=True, stop=True)

        y = [ps.tile([128, HOC], f32, tag=f"y{bt}", name=f"y{bt}")
             for bt in range(2)]
        for bt in range(2):
            nc.tensor.matmul(y[bt][:], ones[0:1, 0:128], brow[0:1, 0:HOC],
                             start=True, stop=False)
        for p in range(NP - 1):
            for c in range(NCH):
                for bt in range(2):
                    nc.tensor.matmul(y[bt][:], plc(p, c, bt),
                                     wAll[:, p, c, :], start=False, stop=False)
        # last plane + packed chunk-6: all of bt0 first so its PSUM bank
        # closes early and the bt0 tail overlaps bt1's matmuls
        for bt in range(2):
            for c in range(NCH):
                nc.tensor.matmul(y[bt][:], plc(NP - 1, c, bt),
                                 wAll[:, NP - 1, c, :], start=False, stop=False)
            nc.tensor.matmul(y[bt][:], f6[:, bt * 128:(bt + 1) * 128],
                             w6t[:], start=False, stop=True)

        # ---- tail per batch-tile: tanh, transpose, blockdiag MLP ----
        lgs = sb.tile([128, 2, 5], f32, tag="lgs")
        for bt in range(2):
            h1 = sb.tile([128, HOC], f16, tag=f"h1{bt}", name=f"h1{bt}")
            nc.scalar.activation(h1[:, 0:128], y[bt][:, 0:128], AF.Tanh)
            nc.scalar.activation(h1[:, 128:], y[bt][:, 128:], AF.Tanh)
            sts = []
            for k in range(3):
                kk = 128 if k < 2 else 64
                pt = ps.tile([128, 128], f16, tag=f"pt{k}",
                             name=f"pt{bt}{k}")
                nc.tensor.transpose(pt[0:kk, :], h1[:, k * 128:k * 128 + kk],
                                    idt)
                st = sb.tile([128, 128], f16, tag=f"st{bt}{k}",
                             name=f"st{bt}{k}")
                nc.vector.tensor_copy(st[0:kk, :], pt[0:kk, :])
                sts.append(st)
            ps2 = ps.tile([128, D2C], f32, tag=f"ps2{bt}", name=f"ps2{bt}")
            nc.tensor.matmul(ps2[:], ones[0:1, 0:128], brow[0:1, HOC:],
                             start=True, stop=False)
            for k in range(3):
                kk = 128 if k < 2 else 64
                nc.tensor.matmul(ps2[:], sts[k][0:kk, :], w1p[0:kk, k, :],
                                 start=False, stop=(k == 2))
            h2 = sb.tile([128, D2C], f32, tag=f"h2{bt}", name=f"h2{bt}")
            nc.scalar.activation(h2[:], ps2[:], AF.Tanh)
            prod = sb.tile([128, D2C], f32, tag=f"prod{bt}", name=f"prod{bt}")
            nc.vector.tensor_tensor(prod[:], h2[:], w2b, op=ALU.mult)
            red = sb.tile([128, 5], f32, tag=f"red{bt}", name=f"red{bt}")
            nc.vector.tensor_reduce(
                red[:], prod[:].rearrange("p (h d) -> p h d", d=32),
                axis=mybir.AxisListType.X, op=ALU.add)
            nc.vector.tensor_tensor(lgs[:, bt, :], red[:], b2b, op=ALU.add)
        # single out DMA: src (p, bt, col) -> dram row bt*128+p
        dst = bass.AP(tensor=out_d.tensor, offset=0,
                      ap=[[5, 128], [128 * 5, 2], [1, 5]])
        nc.sync.dma_start(dst, lgs[:])

    nc.compile()
    _CACHE["nc"] = nc
    return nc


def _prep_inputs(x, coef, scale_base, scale_sp, lmd, W1, b1, W2, b2):
    polyc, tapS, tapR = _tables()
    xf = np.asarray(x, np.float32).reshape(B, I)

    coef = np.asarray(coef, np.float64)
    eff = coef * np.asarray(scale_sp, np.float64)[..., None] \
        * np.asarray(lmd, np.float64)[:, :, None, None]        # (H, I, O, 8)
    W = eff.transpose(1, 3, 0, 2).reshape(I, 8, H * O)         # (I, 8, 640)
    sbl = (np.asarray(scale_base, np.float64)
           * np.asarray(lmd, np.float64)[:, :, None]
           ).transpose(1, 0, 2).reshape(I, H * O)

    # silu(x) lies (to ~1e-6) in the span of the 8-fn spline basis: fit it
    # and fold sbl * beta into the plane weights -- no silu plane on device
    g = np.linspace(0.0, 1.0, 4097)[:-1]
    gc = g - 0.5
    phi = np.stack([np.ones_like(g), gc, gc**2, gc**3,
                    np.maximum(g - 0.6, 0)**3, np.maximum(g - 0.8, 0)**3,
                    np.maximum(0.2 - g, 0)**3, np.maximum(0.4 - g, 0)**3], 1)
    beta = np.linalg.lstsq(phi, g / (1 + np.exp(-g)), rcond=None)[0]

    # fold: device plane order xc, xc^2, xc^3, R3', R4', S1', S2'
    # (d = 5*xc, so d-basis folds scale by 5^s; cubes by 125)
    Wp = np.empty((I, NP, H * O))
    Wp[:, 0] = 5.0 * np.einsum('j,ijo->io', polyc[:, 1], W) + beta[1] * sbl
    Wp[:, 1] = 25.0 * np.einsum('j,ijo->io', polyc[:, 2], W) + beta[2] * sbl
    Wp[:, 2] = 125.0 * np.einsum('j,ijo->io', polyc[:, 3], W) + beta[3] * sbl
    Wp[:, 3] = 125.0 * np.einsum('j,ijo->io', tapR[:, 0], W) + beta[4] * sbl
    Wp[:, 4] = 125.0 * np.einsum('j,ijo->io', tapR[:, 1], W) + beta[5] * sbl
    Wp[:, 5] = 125.0 * np.einsum('j,ijo->io', tapS[:, 0], W) + beta[6] * sbl
    Wp[:, 6] = 125.0 * np.einsum('j,ijo->io', tapS[:, 1], W) + beta[7] * sbl
    bias_full = np.einsum('j,ijo->o', polyc[:, 0], W) \
        + beta[0] * sbl.sum(0)                                 # (640,)

    W1 = np.asarray(W1, np.float64)
    W2 = np.asarray(W2, np.float64).reshape(H * 32)
    b1 = np.asarray(b1, np.float64).reshape(H * 32)
    b2 = np.asarray(b2, np.float64).reshape(H)

    per_og = []
    for og in range(OG):
        hs = slice(og * HOC, (og + 1) * HOC)
        # weight stream: 8 plane pieces [128, 6*320] then packed chunk-6
        pieces = []
        for p in range(NP):
            blk = Wp[0:NCH * 128, p, hs].reshape(NCH, 128, HOC)
            pieces.append(np.ascontiguousarray(
                blk.transpose(1, 0, 2)).reshape(-1))
        w6 = np.zeros((NP * PL, HOC))
        for p in range(NP):
            w6[p * PL:(p + 1) * PL] = Wp[NCH * 128:I, p, hs]
        pieces.append(np.ascontiguousarray(w6).reshape(-1))
        wdev = np.concatenate(pieces).astype(np.float16)

        brow = np.zeros((1, HOC + D2C))
        brow[0, 0:HOC] = bias_full[hs]
        brow[0, HOC:] = b1[og * D2C:(og + 1) * D2C]
        brow = brow.astype(np.float16)

        w1bd = np.zeros((HOC, D2C))
        for hl in range(H // OG):
            w1bd[hl * O:(hl + 1) * O, hl * 32:(hl + 1) * 32] = W1[og * (H // OG) + hl]
        w1dev = np.zeros((128, 3, D2C))
        w1dev[:, 0] = w1bd[0:128]
        w1dev[:, 1] = w1bd[128:256]
        w1dev[0:64, 2] = w1bd[256:HOC]
        cf16 = np.concatenate([w1dev.reshape(128, 3 * D2C),
                               np.eye(128)], 1).astype(np.float16)
        cf32 = np.concatenate([
            np.broadcast_to(W2[og * D2C:(og + 1) * D2C], (128, D2C)),
            np.broadcast_to(b2[og * 5:(og + 1) * 5], (128, 5))],
            1).astype(np.float32)
        per_og.append((wdev, brow, cf16, cf32))

    in_maps = []
    for core in range(NC):
        bg, og = core % BG, core // BG
        xs = (xf[bg * BC:(bg + 1) * BC].T - 0.5).astype(np.float16)  # (784, 256)
        xdev = np.zeros((7, 128, BC), np.float16)
        xdev.reshape(7 * 128, BC)[0:I] = xs
        xdev = np.ascontiguousarray(xdev.transpose(1, 0, 2)).reshape(128, 7 * BC)
        wdev, brow, cf16, cf32 = per_og[og]
        in_maps.append({"x": xdev, "w": wdev, "brow": brow,
                        "cf16": cf16, "cf32": cf32})
    return in_maps


def run(inputs, trace=False, tmpdir=None):
    _install_ntff_hook()
    from concourse.bass_utils import run_bass_kernel_spmd
    nc = _build()
    in_maps = _prep_inputs(**inputs)
    res = run_bass_kernel_spmd(nc, in_maps, core_ids=list(range(NC)),
                               trace=trace, tmpdir=tmpdir)
    out = np.empty((B, H), np.float32)
    for core in range(NC):
        bg, og = core % BG, core // BG
        out[bg * BC:(bg + 1) * BC, og * 5:(og + 1) * 5] = res.results[core]["out"]
    return out, res


def kernel(**inputs):
    out, _ = run(inputs)
    return out
